# revision 1
# baseline (speedup 1.0000x reference)
import os
import sys
from contextlib import ExitStack

import numpy as np

for _p in ("/opt/trn_rl_repo", "/root/.axon_site/_ro/trn_rl_repo"):
    if os.path.isdir(_p) and _p not in sys.path:
        sys.path.insert(0, _p)

import concourse.bass as bass
import concourse.tile as tile
from concourse import bacc
from concourse import mybir
from concourse import bass_utils
from concourse.masks import make_identity

N_CORES = 8
B = 2
N = 2048
C = 1024
H_TOT = 16
D = 64
H_LOC = 4
PAIRS = 2
INNER_LOC = H_LOC * D
M = N // H_TOT
CT = C // 128
NT = N // 128
SCALE = D ** -0.5
FP = mybir.dt.float32
FR = mybir.dt.float32r


def _r(ap):
    return ap.bitcast(FR)


def _build_kernel():
    nc = bacc.Bacc("TRN2", target_bir_lowering=False, debug=False)
    x = nc.dram_tensor("x", (N, C), FP, kind="ExternalInput").ap()
    wq = nc.dram_tensor("wq", (C, INNER_LOC), FP, kind="ExternalInput").ap()
    wk = nc.dram_tensor("wk", (C, INNER_LOC), FP, kind="ExternalInput").ap()
    wv = nc.dram_tensor("wv", (C, INNER_LOC), FP, kind="ExternalInput").ap()
    wo = nc.dram_tensor("wo", (C, C), FP, kind="ExternalInput").ap()
    bo = nc.dram_tensor("bo", (1, C), FP, kind="ExternalInput").ap()
    out = nc.dram_tensor("out", (H_LOC, M, C), FP, kind="ExternalOutput").ap()

    with tile.TileContext(nc) as tc:
        _trace_kernel(tc, out, x, wq, wk, wv, wo, bo)
    nc.compile()
    return nc


def _trace_kernel(tc, out, x, wq, wk, wv, wo, bo):
    nc = tc.nc
    Exp = mybir.ActivationFunctionType.Exp
    Copy = mybir.ActivationFunctionType.Copy

    with ExitStack() as ctx:
        consts = ctx.enter_context(tc.tile_pool(name="consts", bufs=1))
        qkv_pool = ctx.enter_context(tc.tile_pool(name="qkv", bufs=1))
        pt_pool = ctx.enter_context(tc.tile_pool(name="pt", bufs=5))

        ones_t = consts.tile([128, 128], FP)
        nc.vector.memset(ones_t, 1.0)
        bo_raw = consts.tile([1, C], FP)
        nc.sync.dma_start(out=bo_raw, in_=bo)
        ones_fr = consts.tile([1, 128], FR)
        nc.vector.tensor_copy(ones_fr, ones_t[0:1, :])
        bo_sb = consts.tile([1, C], FR)
        nc.vector.tensor_copy(bo_sb, bo_raw)

        qT = qkv_pool.tile([128, PAIRS, N], FR)
        kT = qkv_pool.tile([128, PAIRS, N], FR)
        v_sb = qkv_pool.tile([128, NT, H_LOC, 128], FR)
        nc.vector.tensor_copy(
            v_sb[:, :, :, D:],
            ones_t[:, 0:1].broadcast_to([128, NT, H_LOC, D]),
        )

        s_ps_pool = ctx.enter_context(
            tc.tile_pool(name="s_ps", bufs=2, space="PSUM")
        )
        o_ps_pool = ctx.enter_context(
            tc.tile_pool(name="o_ps", bufs=2, space="PSUM")
        )

        def attn_block(p, ic, jts):
            i0 = ic * 1024
            for jt in jts:
                s_ps = [
                    s_ps_pool.tile([128, 1024], FP, tag="s", name="s_ps")
                    for _ in range(2)
                ]
                for e in range(2):
                    r0, r1 = e * 64, (e + 1) * 64
                    for sub in range(2):
                        nc.tensor.matmul(
                            s_ps[e][:, sub * 512 : (sub + 1) * 512],
                            lhsT=kT[r0:r1, p, jt * 128 : (jt + 1) * 128],
                            rhs=qT[
                                r0:r1, p, i0 + sub * 512 : i0 + (sub + 1) * 512
                            ],
                            start=True,
                            stop=True,
                            tile_position=(e * 64, 0),
                        )
                pts = []
                for e in range(2):
                    pt = pt_pool.tile([128, 1024], FR, tag="pt", name="pt")
                    nc.scalar.activation(
                        out=pt, in_=s_ps[e], func=Exp, scale=SCALE
                    )
                    pts.append(pt)
                for e in range(2):
                    h = 2 * p + e
                    for sub in range(2):
                        nc.tensor.matmul(
                            o_ps[p][ic][e][:, sub * 512 : (sub + 1) * 512],
                            lhsT=v_sb[:, jt, h, :],
                            rhs=pts[e][:, sub * 512 : (sub + 1) * 512],
                            start=(jt == 0),
                            stop=(jt == NT - 1),
                        )

        o_ps = [[None, None], [None, None]]

        with ExitStack() as ictx:
            xpool = ictx.enter_context(tc.tile_pool(name="xtiles", bufs=5))
            wstage = ictx.enter_context(tc.tile_pool(name="wstage", bufs=3))
            wpool = ictx.enter_context(tc.tile_pool(name="wtiles", bufs=1))
            xT_pool = ictx.enter_context(tc.tile_pool(name="xT", bufs=1))

            ident = wpool.tile([128, 128], FP)
            make_identity(nc, ident)
            xT = xT_pool.tile([128, CT, N], FR)
            wq_sb = wpool.tile([128, CT, INNER_LOC], FR)
            wk_sb = wpool.tile([128, CT, INNER_LOC], FR)
            wv_sb = wpool.tile([128, CT, INNER_LOC], FR)

            def emit_group_a(ng):
                xts = []
                for k in range(4):
                    nt = ng * 4 + k
                    x_t = xpool.tile([128, C], FP, tag="x_t", name="x_t")
                    nc.sync.dma_start(
                        out=x_t, in_=x[nt * 128 : (nt + 1) * 128, :]
                    )
                    xts.append(x_t)
                if ng == 0:
                    for ct in range(CT):
                        for wdram, wsb in (
                            (wq, wq_sb),
                            (wk, wk_sb),
                            (wv, wv_sb),
                        ):
                            wst = wstage.tile(
                                [128, INNER_LOC], FP, tag="wst", name="wst"
                            )
                            nc.sync.dma_start(
                                out=wst, in_=wdram[ct * 128 : (ct + 1) * 128, :]
                            )
                            nc.vector.tensor_copy(wsb[:, ct, :], wst)
                for ct in range(CT):
                    ps_t = s_ps_pool.tile([128, 512], FP, tag="s", name="ps_t")
                    for k in range(4):
                        nc.tensor.transpose(
                            ps_t[:, k * 128 : (k + 1) * 128],
                            xts[k][:, ct * 128 : (ct + 1) * 128],
                            ident,
                        )
                    nc.vector.tensor_copy(
                        xT[:, ct, ng * 512 : (ng + 1) * 512], ps_t
                    )

            def emit_group_b(ng):
                for p in range(PAIRS):
                    for dst, wsb in ((qT, wq_sb), (kT, wk_sb)):
                        ps_p = s_ps_pool.tile(
                            [128, 512], FP, tag="s", name="ps_p"
                        )
                        for ct in range(CT):
                            nc.tensor.matmul(
                                ps_p,
                                lhsT=wsb[:, ct, p * 128 : (p + 1) * 128],
                                rhs=xT[:, ct, ng * 512 : (ng + 1) * 512],
                                start=(ct == 0),
                                stop=(ct == CT - 1),
                            )
                        nc.vector.tensor_copy(
                            dst[:, p, ng * 512 : (ng + 1) * 512], ps_p
                        )
                for k in range(4):
                    nt = ng * 4 + k
                    ps_v = s_ps_pool.tile([128, 512], FP, tag="s", name="ps_v")
                    for ct in range(CT):
                        nc.tensor.matmul(
                            ps_v[:, 0:INNER_LOC],
                            lhsT=xT[:, ct, nt * 128 : (nt + 1) * 128],
                            rhs=wv_sb[:, ct, :],
                            start=(ct == 0),
                            stop=(ct == CT - 1),
                        )
                    nc.vector.tensor_copy(
                        v_sb[:, nt, :, 0:D],
                        ps_v[:, 0:INNER_LOC].rearrange(
                            "p (h d) -> p h d", h=H_LOC
                        ),
                    )

            emit_group_a(0)
            emit_group_b(0)
            emit_group_a(1)
            emit_group_b(1)
            o_ps[0][0] = [
                o_ps_pool.tile([128, 1024], FP, tag="o", name="o_ps")
                for _ in range(2)
            ]
            attn_block(0, 0, range(0, 2))
            emit_group_a(2)
            attn_block(0, 0, range(2, 4))
            emit_group_b(2)
            attn_block(0, 0, range(4, 6))
            emit_group_a(3)
            attn_block(0, 0, range(6, 8))
            emit_group_b(3)

        attn_block(0, 0, range(8, 16))

        lt_pool = ctx.enter_context(tc.tile_pool(name="lt", bufs=1))
        ou_pool = ctx.enter_context(tc.tile_pool(name="ou", bufs=2))
        rec_pool = ctx.enter_context(tc.tile_pool(name="rec", bufs=2))
        rb_pool = ctx.enter_context(tc.tile_pool(name="rb", bufs=3))
        out_pool = ctx.enter_context(tc.tile_pool(name="outsb", bufs=2))
        wo_ring = ctx.enter_context(tc.tile_pool(name="wo_ring", bufs=3))

        LT128 = lt_pool.tile([128, H_LOC, 8, M], FR)
        LTodd = lt_pool.tile([64, H_LOC, 8, M], FR)

        def norm_block(p, ic):
            for e in range(2):
                h = 2 * p + e
                ou_t = ou_pool.tile([128, 1024], FP, tag="ou", name="ou_t")
                nc.vector.tensor_copy(ou_t, o_ps[p][ic][e])
                rec_t = rec_pool.tile([128, 1024], FP, tag="rec", name="rec_t")
                nc.vector.reciprocal(
                    out=rec_t[64:128, :], in_=ou_t[64:128, :]
                )
                rb_t = rb_pool.tile([64, 1024], FP, tag="rb", name="rb_t")
                nc.sync.dma_start(out=rb_t, in_=rec_t[64:128, :])
                ou_w = ou_t.rearrange("q (w m) -> q w m", w=8)
                rb_w = rb_t.rearrange("q (w m) -> q w m", w=8)
                kts = slice(4 * ic, 4 * ic + 4)
                nc.vector.tensor_mul(
                    LT128[0:64, h, kts, :],
                    ou_w[0:64, 0::2, :],
                    rb_w[:, 0::2, :],
                )
                nc.vector.tensor_mul(
                    LTodd[:, h, kts, :],
                    ou_w[0:64, 1::2, :],
                    rb_w[:, 1::2, :],
                )
                nc.sync.dma_start(
                    out=LT128[64:128, h, kts, :],
                    in_=LTodd[:, h, kts, :],
                )

        norm_block(0, 0)
        for p, ic in ((0, 1), (1, 0), (1, 1)):
            o_ps[p][ic] = [
                o_ps_pool.tile([128, 1024], FP, tag="o", name="o_ps")
                for _ in range(2)
            ]
            attn_block(p, ic, range(NT))
            norm_block(p, ic)

        f_tiles = [
            s_ps_pool.tile([128, 1024], FP, tag="s", name="f_s0"),
            s_ps_pool.tile([128, 1024], FP, tag="s", name="f_s1"),
            o_ps_pool.tile([128, 1024], FP, tag="o", name="f_o0"),
            o_ps_pool.tile([128, 1024], FP, tag="o", name="f_o1"),
        ]
        f_ps = [
            f_tiles[i // 2][:, (i % 2) * 512 : (i % 2 + 1) * 512]
            for i in range(8)
        ]
        wo_ts = []
        for kt in range(8):
            wo_raw = wo_ring.tile([128, C], FP, tag="wo_raw", name="wo_raw")
            nc.sync.dma_start(out=wo_raw, in_=wo[kt * 128 : (kt + 1) * 128, :])
            wo_t = wo_ring.tile([128, C], FR, tag="wo_t", name=f"wo_t{kt}")
            nc.vector.tensor_copy(wo_t, wo_raw)
            wo_ts.append(wo_t)
            if kt == 7:
                break
            for h in range(H_LOC):
                for oc in range(2):
                    nc.tensor.matmul(
                        f_ps[h * 2 + oc],
                        lhsT=LT128[:, h, kt, :],
                        rhs=wo_t[:, oc * 512 : (oc + 1) * 512],
                        start=(kt == 0),
                        stop=False,
                    )
        for h in range(H_LOC):
            for oc in range(2):
                nc.tensor.matmul(
                    f_ps[h * 2 + oc],
                    lhsT=LT128[:, h, 7, :],
                    rhs=wo_ts[7][:, oc * 512 : (oc + 1) * 512],
                    start=False,
                    stop=False,
                )
                nc.tensor.matmul(
                    f_ps[h * 2 + oc],
                    lhsT=ones_fr,
                    rhs=bo_sb[0:1, oc * 512 : (oc + 1) * 512],
                    start=False,
                    stop=True,
                )
                ob = out_pool.tile([128, 512], FP, tag="ob", name="ob")
                nc.scalar.activation(out=ob, in_=f_ps[h * 2 + oc], func=Copy)
                nc.sync.dma_start(
                    out=out[h, :, oc * 512 : (oc + 1) * 512], in_=ob
                )


_NC = None


def _get_nc():
    global _NC
    if _NC is None:
        _NC = _build_kernel()
    return _NC


def _make_in_maps(x, Wq, Wkv, Wo, bo):
    in_maps = []
    for c in range(N_CORES):
        b = c // 4
        g = c % 4
        cols = slice(g * INNER_LOC, (g + 1) * INNER_LOC)
        in_maps.append(
            {
                "x": np.ascontiguousarray(x[b]),
                "wq": np.ascontiguousarray(Wq[:, cols]),
                "wk": np.ascontiguousarray(Wkv[:, cols]),
                "wv": np.ascontiguousarray(
                    Wkv[:, C + g * INNER_LOC : C + (g + 1) * INNER_LOC]
                ),
                "wo": np.ascontiguousarray(Wo),
                "bo": np.ascontiguousarray(bo.reshape(1, C)),
            }
        )
    return in_maps


def _run(x, Wq, Wkv, Wo, bo, **run_kwargs):
    nc = _get_nc()
    in_maps = _make_in_maps(x, Wq, Wkv, Wo, bo)
    res = bass_utils.run_bass_kernel_spmd(
        nc, in_maps, core_ids=list(range(N_CORES)), **run_kwargs
    )
    outs = [res.results[c]["out"].reshape(H_LOC, M, C) for c in range(N_CORES)]
    full = np.concatenate(outs, axis=0).astype(np.float32)
    return full, res


def kernel(x, Wq, Wkv, Wo, bo):
    x = np.asarray(x, dtype=np.float32)
    Wq = np.asarray(Wq, dtype=np.float32)
    Wkv = np.asarray(Wkv, dtype=np.float32)
    Wo = np.asarray(Wo, dtype=np.float32)
    bo = np.asarray(bo, dtype=np.float32)
    full, _ = _run(x, Wq, Wkv, Wo, bo)
    return full



# revision 27
# speedup vs baseline: 1.2113x; 1.2113x over previous
import os
import sys
from contextlib import ExitStack

import numpy as np

for _p in ("/opt/trn_rl_repo", "/root/.axon_site/_ro/trn_rl_repo"):
    if os.path.isdir(_p) and _p not in sys.path:
        sys.path.insert(0, _p)

import concourse.bass as bass
import concourse.tile as tile
from concourse import bacc
from concourse import mybir
from concourse import bass_utils
from concourse.masks import make_identity

N_CORES = 8
B = 2
N = 2048
C = 1024
H_TOT = 16
D = 64
H_LOC = 4
PAIRS = 2
INNER_LOC = H_LOC * D
M = N // H_TOT
CT = C // 128
NT = N // 128
NCH = 4
CHUNK = 512
SCALE = D ** -0.5
FP = mybir.dt.float32
FR = mybir.dt.float32r


def _r(ap):
    return ap.bitcast(FR)


def _build_kernel():
    nc = bacc.Bacc("TRN2", target_bir_lowering=False, debug=False)
    x = nc.dram_tensor("x", (N, C), FP, kind="ExternalInput").ap()
    wq = nc.dram_tensor("wq", (C, INNER_LOC), FP, kind="ExternalInput").ap()
    wk = nc.dram_tensor("wk", (C, INNER_LOC), FP, kind="ExternalInput").ap()
    wv = nc.dram_tensor("wv", (C, INNER_LOC), FP, kind="ExternalInput").ap()
    wo = nc.dram_tensor("wo", (C, C), FP, kind="ExternalInput").ap()
    bo = nc.dram_tensor("bo", (1, C), FP, kind="ExternalInput").ap()
    out = nc.dram_tensor("out", (H_LOC, M, C), FP, kind="ExternalOutput").ap()

    with tile.TileContext(nc) as tc:
        _trace_kernel(tc, out, x, wq, wk, wv, wo, bo)
    nc.compile()
    return nc


def _trace_kernel(tc, out, x, wq, wk, wv, wo, bo):
    nc = tc.nc
    Exp = mybir.ActivationFunctionType.Exp
    Copy = mybir.ActivationFunctionType.Copy
    BF = mybir.dt.bfloat16
    LAG = 6
    NB = 128

    with ExitStack() as ctx:
        consts = ctx.enter_context(tc.tile_pool(name="consts", bufs=1))
        qkv_pool = ctx.enter_context(tc.tile_pool(name="qkv", bufs=1))
        lt_pool = ctx.enter_context(tc.tile_pool(name="lt", bufs=1))
        lodd_pool = ctx.enter_context(tc.tile_pool(name="lodd", bufs=1))
        pt_pool = ctx.enter_context(tc.tile_pool(name="pt", bufs=LAG + 2))
        ou_pool = ctx.enter_context(tc.tile_pool(name="ou", bufs=2))
        rb_pool = ctx.enter_context(tc.tile_pool(name="rb", bufs=2))
        out_pool = ctx.enter_context(tc.tile_pool(name="outsb", bufs=2))

        s_pool = ctx.enter_context(
            tc.tile_pool(name="s_ps", bufs=2, space="PSUM")
        )
        o_pool = ctx.enter_context(
            tc.tile_pool(name="o_ps", bufs=1, space="PSUM")
        )
        sc_pool = ctx.enter_context(
            tc.tile_pool(name="sc_ps", bufs=2, space="PSUM")
        )

        ones_t = consts.tile([128, 128], FP)
        nc.vector.memset(ones_t, 1.0)
        bo_raw = consts.tile([1, C], FP)
        bo_fr = consts.tile([1, C], FR)
        ones_fr = consts.tile([1, 128], FR)
        nc.vector.tensor_copy(ones_fr, ones_t[0:1, :])
        warm = consts.tile([1, 128], FR)
        nc.scalar.activation(out=warm, in_=ones_t[0:1, :], func=Exp, scale=1.0)

        qT = qkv_pool.tile([128, PAIRS, N], FR)
        kT = qkv_pool.tile([128, PAIRS, N], FR)
        v_sb = qkv_pool.tile([128, NT, H_LOC, 128], BF)
        nc.vector.tensor_copy(
            v_sb[:, :, :, D:],
            ones_t[:, 0:1].broadcast_to([128, NT, H_LOC, D]),
        )

        LTt = {
            (h, cp): lt_pool.tile([128, 2, M], FR, name=f"lt_{h}_{cp}")
            for h in range(H_LOC)
            for cp in range(NCH)
        }

        pts = {}

        def u_S(p, c, jt):
            s_t = s_pool.tile([128, 1024], FP, tag="s", name="s_t")
            i0 = c * CHUNK
            for e in range(2):
                r0, r1 = e * 64, (e + 1) * 64
                nc.tensor.matmul(
                    s_t[:, e * 512 : (e + 1) * 512],
                    lhsT=kT[r0:r1, p, jt * 128 : (jt + 1) * 128],
                    rhs=qT[r0:r1, p, i0 : i0 + CHUNK],
                    start=True,
                    stop=True,
                    tile_position=(e * 64, 0),
                )
            pt = pt_pool.tile([128, 1024], BF, tag="pt", name="pt")
            nc.scalar.activation(out=pt, in_=s_t, func=Exp, scale=SCALE)
            pts[(p, c, jt)] = pt

        def u_PV(p, c, jt, o_t):
            pt = pts.pop((p, c, jt))
            for e in range(2):
                h = 2 * p + e
                nc.tensor.matmul(
                    o_t[:, e * 512 : (e + 1) * 512],
                    lhsT=v_sb[:, jt, h, :],
                    rhs=pt[:, e * 512 : (e + 1) * 512],
                    start=(jt == 0),
                    stop=(jt == NT - 1),
                )

        def u_norm(p, c, o_t, act_ou=False):
            for e in range(2):
                h = 2 * p + e
                ou_t = ou_pool.tile([128, 512], FP, tag="ou", name="ou_t")
                if act_ou:
                    nc.scalar.activation(
                        out=ou_t,
                        in_=o_t[:, e * 512 : (e + 1) * 512],
                        func=Copy,
                    )
                else:
                    nc.vector.tensor_copy(
                        ou_t, o_t[:, e * 512 : (e + 1) * 512]
                    )
                nc.vector.reciprocal(
                    out=ou_t[64:128, :], in_=ou_t[64:128, :]
                )
                rb_t = rb_pool.tile([64, 512], FP, tag="rb", name="rb_t")
                nc.sync.dma_start(out=rb_t, in_=ou_t[64:128, :])
                ou_w = ou_t.rearrange("q (w m) -> q w m", w=4)
                rb_w = rb_t.rearrange("q (w m) -> q w m", w=4)
                lodd_t = lodd_pool.tile([64, 2, M], FR, tag="lo", name="lo")
                nc.vector.tensor_mul(
                    LTt[(h, c)][0:64, :, :],
                    ou_w[0:64, 0::2, :],
                    rb_w[:, 0::2, :],
                )
                nc.vector.tensor_mul(
                    lodd_t,
                    ou_w[0:64, 1::2, :],
                    rb_w[:, 1::2, :],
                )
                nc.sync.dma_start(
                    out=LTt[(h, c)][64:128, :, :],
                    in_=lodd_t,
                )

        o_tiles = {}

        def blk(g):
            return (g // 64, (g // 16) % 4, g % 16)

        def run_stream(g0, g1, fillers):
            for g in range(g0, g1):
                if g + 1 < NB:
                    u_S(*blk(g + 1))
                for f in fillers.get(g, ()):
                    f()
                pv = g - LAG
                if 0 <= pv < NB:
                    p, c, jt = blk(pv)
                    if jt == 0:
                        o_tiles[(p, c)] = o_pool.tile(
                            [128, 1024], FP, tag="o", name="o_t"
                        )
                    u_PV(p, c, jt, o_tiles[(p, c)])
                    if jt == NT - 1:
                        u_norm(
                            p, c, o_tiles.pop((p, c)), act_ou=(pv == NB - 1)
                        )

        with ExitStack() as pctx:
            pro = pctx.enter_context(tc.tile_pool(name="pro", bufs=1))
            xpool = pctx.enter_context(tc.tile_pool(name="xtiles", bufs=8))

            ident = pro.tile([128, 128], FP)
            make_identity(nc, ident)
            xTr = pro.tile([128, CT, 2, CHUNK], FR)
            wq_sb = pro.tile([128, CT, INNER_LOC], FR)
            wk_sb = pro.tile([128, CT, INNER_LOC], FR)
            wv_sb = pro.tile([128, CT, INNER_LOC], FR)
            wraw_pool = pctx.enter_context(
                tc.tile_pool(name="wraw", bufs=2)
            )
            XSL = (0, 1, 0, 1)

            def u_wload(wdram, wsb):
                wr = wraw_pool.tile(
                    [128, CT, INNER_LOC], FP, tag="wr", name="wr"
                )
                nc.sync.dma_start(
                    out=wr, in_=wdram.rearrange("(t r) f -> r t f", r=128)
                )
                nc.scalar.activation(out=wsb, in_=wr, func=Copy)

            xts = {}

            def u_xload(ng):
                for k in range(4):
                    nt = ng * 4 + k
                    x_t = xpool.tile([128, C], FP, tag="x_t", name="x_t")
                    nc.sync.dma_start(
                        out=x_t, in_=x[nt * 128 : (nt + 1) * 128, :]
                    )
                    xts[nt] = x_t

            def u_T(ng, cts):
                for ct in cts:
                    ps_t = sc_pool.tile([128, 512], FP, tag="sc", name="ps_t")
                    for k in range(4):
                        nt = ng * 4 + k
                        nc.tensor.transpose(
                            ps_t[:, k * 128 : (k + 1) * 128],
                            xts[nt][:, ct * 128 : (ct + 1) * 128],
                            ident,
                        )
                    nc.vector.tensor_copy(xTr[:, ct, XSL[ng], :], ps_t)

            def u_QK(ng, p, dst, wsb):
                ps_p = sc_pool.tile([128, 512], FP, tag="sc", name="ps_p")
                for ct in range(CT):
                    nc.tensor.matmul(
                        ps_p,
                        lhsT=wsb[:, ct, p * 128 : (p + 1) * 128],
                        rhs=xTr[:, ct, XSL[ng], :],
                        start=(ct == 0),
                        stop=(ct == CT - 1),
                    )
                nc.vector.tensor_copy(
                    dst[:, p, ng * 512 : (ng + 1) * 512], ps_p
                )

            def u_V(ng, k):
                nt = ng * 4 + k
                ps_v = sc_pool.tile([128, 512], FP, tag="sc", name="ps_v")
                for ct in range(CT):
                    nc.tensor.matmul(
                        ps_v[:, 0:INNER_LOC],
                        lhsT=xTr[:, ct, XSL[ng], k * 128 : (k + 1) * 128],
                        rhs=wv_sb[:, ct, :],
                        start=(ct == 0),
                        stop=(ct == CT - 1),
                    )
                nc.vector.tensor_copy(
                    v_sb[:, nt, :, 0:D],
                    ps_v[:, 0:INNER_LOC].rearrange(
                        "p (h d) -> p h d", h=H_LOC
                    ),
                )

            u_xload(0)
            u_wload(wq, wq_sb)
            u_wload(wk, wk_sb)
            u_xload(1)
            u_wload(wv, wv_sb)
            nc.sync.dma_start(out=bo_raw, in_=bo)
            nc.vector.tensor_copy(bo_fr, bo_raw)

            u_T(0, [0, 1, 2, 3])
            u_T(0, [4, 5, 6, 7])
            u_QK(0, 0, qT, wq_sb)
            u_QK(0, 0, kT, wk_sb)
            u_xload(2)
            u_S(0, 0, 0)

            pro_fillers = {
                1: [lambda: u_T(1, [0, 1, 2, 3])],
                2: [
                    lambda: u_T(1, [4, 5, 6, 7]),
                    lambda: u_QK(1, 0, kT, wk_sb),
                ],
                3: [
                    lambda: u_xload(3),
                    lambda: u_QK(0, 1, qT, wq_sb),
                ],
                4: [lambda: u_QK(0, 1, kT, wk_sb)],
                5: [
                    lambda: u_V(0, 0),
                    lambda: u_V(0, 1),
                    lambda: u_V(0, 2),
                    lambda: u_V(0, 3),
                ],
                6: [
                    lambda: u_T(2, [0, 1, 2, 3]),
                    lambda: u_T(2, [4, 5, 6, 7]),
                    lambda: u_QK(2, 0, kT, wk_sb),
                ],
                8: [
                    lambda: u_V(1, 0),
                    lambda: u_V(1, 1),
                    lambda: u_QK(1, 0, qT, wq_sb),
                ],
                9: [
                    lambda: u_V(1, 2),
                    lambda: u_V(1, 3),
                    lambda: u_QK(1, 1, qT, wq_sb),
                    lambda: u_QK(1, 1, kT, wk_sb),
                ],
                10: [
                    lambda: u_T(3, [0, 1, 2, 3]),
                    lambda: u_T(3, [4, 5, 6, 7]),
                    lambda: u_QK(3, 0, kT, wk_sb),
                ],
                11: [lambda: u_V(2, 0), lambda: u_V(2, 1)],
                12: [lambda: u_V(2, 2), lambda: u_V(2, 3)],
                13: [lambda: u_V(3, 0), lambda: u_V(3, 1)],
                14: [lambda: u_V(3, 2)],
                15: [lambda: u_V(3, 3)],
                28: [lambda: u_QK(2, 0, qT, wq_sb)],
                34: [lambda: u_QK(2, 1, qT, wq_sb)],
                38: [lambda: u_QK(2, 1, kT, wk_sb)],
                42: [lambda: u_QK(3, 0, qT, wq_sb)],
                50: [lambda: u_QK(3, 1, qT, wq_sb)],
                56: [lambda: u_QK(3, 1, kT, wk_sb)],
            }
            run_stream(0, 57, pro_fillers)

        wo_pool = ctx.enter_context(tc.tile_pool(name="wo_sbuf", bufs=1))
        wo_ring = ctx.enter_context(tc.tile_pool(name="wo_ring", bufs=2))
        wo_sb = wo_pool.tile([128, CT, C], FR)
        for kt in range(CT):
            wo_raw = wo_ring.tile([128, C], FP, tag="wor", name="wo_raw")
            nc.sync.dma_start(
                out=wo_raw, in_=wo[kt * 128 : (kt + 1) * 128, :]
            )
            nc.vector.tensor_copy(wo_sb[:, kt, :], wo_raw)

        def u_Fk(h, oc, kt, f_ps, first=False):
            return nc.tensor.matmul(
                f_ps,
                lhsT=LTt[(h, kt // 2)][:, kt % 2, :],
                rhs=wo_sb[:, kt, oc * 512 : (oc + 1) * 512],
                start=first,
                stop=False,
            )

        def u_Fend(h, oc, f_ps, dve=False):
            nc.tensor.matmul(
                f_ps,
                lhsT=ones_fr,
                rhs=bo_fr[0:1, oc * 512 : (oc + 1) * 512],
                start=False,
                stop=True,
            )
            ob = out_pool.tile([128, 512], FP, tag="ob", name="ob")
            if dve:
                nc.vector.tensor_copy(ob, f_ps)
            else:
                nc.scalar.activation(out=ob, in_=f_ps, func=Copy)
            nc.sync.dma_start(
                out=out[h, :, oc * 512 : (oc + 1) * 512], in_=ob
            )

        f_tiles = {}

        def f_ps_of(h, oc):
            if (h, oc) not in f_tiles:
                f_tiles[(h, oc)] = sc_pool.tile(
                    [128, 512], FP, tag="sc", name=f"f_{h}_{oc}"
                )
            return f_tiles[(h, oc)]

        def fk(h, oc, kt, first=False):
            return lambda: u_Fk(h, oc, kt, f_ps_of(h, oc), first)

        def fend(h, oc):
            return lambda: u_Fend(h, oc, f_ps_of(h, oc))

        def _g0(kt):
            return 16 * (kt // 2 + 1) + LAG + 1

        chain_a = (
            [(fk(0, 0, kt, kt == 0), _g0(kt)) for kt in range(8)]
            + [(fend(0, 0), _g0(7))]
            + [(fk(1, 0, kt, kt == 0), _g0(kt)) for kt in range(8)]
            + [(fend(1, 0), _g0(7))]
        )
        chain_b = (
            [(fk(0, 1, kt, kt == 0), _g0(kt)) for kt in range(8)]
            + [(fend(0, 1), _g0(7))]
            + [(fk(1, 1, kt, kt == 0), _g0(kt)) for kt in range(8)]
            + [(fend(1, 1), _g0(7))]
        )
        reserve_a = [chain_a.pop()[0] for _ in range(4)][::-1]
        reserve_b = [chain_b.pop()[0] for _ in range(4)][::-1]
        p1_fillers = {}
        turn = [chain_a, chain_b]
        for g in range(57, 128):
            lst = []
            for chain in turn:
                if len(lst) >= (2 if g % 16 >= 14 else 1):
                    break
                if chain and chain[0][1] <= g:
                    lst.append(chain.pop(0)[0])
            turn.reverse()
            if lst:
                p1_fillers[g] = lst
        run_stream(57, NB + LAG, p1_fillers)

        for fcl, _g in chain_a + chain_b:
            fcl()
        f3a = s_pool.tile([128, 1024], FP, tag="s", name="f3a")
        f3b = s_pool.tile([128, 1024], FP, tag="s", name="f3b")
        f_tiles[(3, 0)] = f3a[:, 0:512]
        f_tiles[(3, 1)] = f3b[:, 0:512]
        for kt in range(6):
            u_Fk(3, 0, kt, f_tiles[(3, 0)], first=(kt == 0))
            u_Fk(3, 1, kt, f_tiles[(3, 1)], first=(kt == 0))
        for fcl in reserve_a + reserve_b:
            fcl()
        for kt in range(6):
            u_Fk(2, 0, kt, f_ps_of(2, 0), first=(kt == 0))
            u_Fk(2, 1, kt, f_ps_of(2, 1), first=(kt == 0))
        for i, (h, oc) in enumerate(((2, 0), (2, 1), (3, 0), (3, 1))):
            u_Fk(h, oc, 6, f_tiles.get((h, oc)) or f_ps_of(h, oc))
            u_Fk(h, oc, 7, f_tiles.get((h, oc)) or f_ps_of(h, oc))
            u_Fend(
                h, oc,
                f_tiles.get((h, oc)) or f_ps_of(h, oc),
                dve=(i % 2 == 1),
            )


_NC = None


def _get_nc():
    global _NC
    if _NC is None:
        _NC = _build_kernel()
    return _NC


def _make_in_maps(x, Wq, Wkv, Wo, bo):
    in_maps = []
    for c in range(N_CORES):
        b = c // 4
        g = c % 4
        cols = slice(g * INNER_LOC, (g + 1) * INNER_LOC)
        in_maps.append(
            {
                "x": np.ascontiguousarray(x[b]),
                "wq": np.ascontiguousarray(Wq[:, cols]),
                "wk": np.ascontiguousarray(Wkv[:, cols]),
                "wv": np.ascontiguousarray(
                    Wkv[:, C + g * INNER_LOC : C + (g + 1) * INNER_LOC]
                ),
                "wo": np.ascontiguousarray(Wo),
                "bo": np.ascontiguousarray(bo.reshape(1, C)),
            }
        )
    return in_maps


def _run(x, Wq, Wkv, Wo, bo, **run_kwargs):
    nc = _get_nc()
    in_maps = _make_in_maps(x, Wq, Wkv, Wo, bo)
    res = bass_utils.run_bass_kernel_spmd(
        nc, in_maps, core_ids=list(range(N_CORES)), **run_kwargs
    )
    outs = [res.results[c]["out"].reshape(H_LOC, M, C) for c in range(N_CORES)]
    full = np.concatenate(outs, axis=0).astype(np.float32)
    return full, res


def kernel(x, Wq, Wkv, Wo, bo):
    x = np.asarray(x, dtype=np.float32)
    Wq = np.asarray(Wq, dtype=np.float32)
    Wkv = np.asarray(Wkv, dtype=np.float32)
    Wo = np.asarray(Wo, dtype=np.float32)
    bo = np.asarray(bo, dtype=np.float32)
    full, _ = _run(x, Wq, Wkv, Wo, bo)
    return full


# revision 38
# speedup vs baseline: 1.2339x; 1.0186x over previous
import os
import sys
from contextlib import ExitStack

import numpy as np

for _p in ("/opt/trn_rl_repo", "/root/.axon_site/_ro/trn_rl_repo"):
    if os.path.isdir(_p) and _p not in sys.path:
        sys.path.insert(0, _p)

import concourse.bass as bass
import concourse.tile as tile
from concourse import bacc
from concourse import mybir
from concourse import bass_utils
from concourse.masks import make_identity

N_CORES = 8
B = 2
N = 2048
C = 1024
H_TOT = 16
D = 64
H_LOC = 4
PAIRS = 2
INNER_LOC = H_LOC * D
M = N // H_TOT
CT = C // 128
NT = N // 128
NCH = 4
CHUNK = 512
SCALE = D ** -0.5
FP = mybir.dt.float32
FR = mybir.dt.float32r


def _r(ap):
    return ap.bitcast(FR)


def _build_kernel():
    nc = bacc.Bacc("TRN2", target_bir_lowering=False, debug=False)
    x = nc.dram_tensor("x", (N, C), FP, kind="ExternalInput").ap()
    wq = nc.dram_tensor("wq", (C, INNER_LOC), FP, kind="ExternalInput").ap()
    wk = nc.dram_tensor("wk", (C, INNER_LOC), FP, kind="ExternalInput").ap()
    wv = nc.dram_tensor("wv", (C, INNER_LOC), FP, kind="ExternalInput").ap()
    wo = nc.dram_tensor("wo", (C, C), FP, kind="ExternalInput").ap()
    bo = nc.dram_tensor("bo", (1, C), FP, kind="ExternalInput").ap()
    out = nc.dram_tensor("out", (H_LOC, M, C), FP, kind="ExternalOutput").ap()

    with tile.TileContext(nc) as tc:
        _trace_kernel(tc, out, x, wq, wk, wv, wo, bo)
    nc.compile()
    return nc


def _trace_kernel(tc, out, x, wq, wk, wv, wo, bo):
    nc = tc.nc
    Exp = mybir.ActivationFunctionType.Exp
    Copy = mybir.ActivationFunctionType.Copy
    BF = mybir.dt.bfloat16
    LAG = 6
    NB = 128

    with ExitStack() as ctx:
        consts = ctx.enter_context(tc.tile_pool(name="consts", bufs=1))
        qkv_pool = ctx.enter_context(tc.tile_pool(name="qkv", bufs=1))
        lt_pool = ctx.enter_context(tc.tile_pool(name="lt", bufs=1))
        lodd_pool = ctx.enter_context(tc.tile_pool(name="lodd", bufs=1))
        pt_pool = ctx.enter_context(tc.tile_pool(name="pt", bufs=LAG + 2))
        ou_pool = ctx.enter_context(tc.tile_pool(name="ou", bufs=2))
        rb_pool = ctx.enter_context(tc.tile_pool(name="rb", bufs=2))
        out_pool = ctx.enter_context(tc.tile_pool(name="outsb", bufs=2))

        s_pool = ctx.enter_context(
            tc.tile_pool(name="s_ps", bufs=2, space="PSUM")
        )
        o_pool = ctx.enter_context(
            tc.tile_pool(name="o_ps", bufs=1, space="PSUM")
        )
        sc_pool = ctx.enter_context(
            tc.tile_pool(name="sc_ps", bufs=2, space="PSUM")
        )

        ones_t = consts.tile([128, 128], FP)
        nc.vector.memset(ones_t, 1.0)
        bo_raw = consts.tile([1, C], FP)
        bo_fr = consts.tile([1, C], FR)
        ones_fr = consts.tile([1, 128], FR)
        nc.vector.tensor_copy(ones_fr, ones_t[0:1, :])
        warm = consts.tile([1, 128], FR)
        nc.scalar.activation(out=warm, in_=ones_t[0:1, :], func=Exp, scale=1.0)

        qT = qkv_pool.tile([128, PAIRS, N], FR)
        kT = qkv_pool.tile([128, PAIRS, N], FR)
        v_sb = qkv_pool.tile([128, NT, H_LOC, 128], BF)
        nc.vector.tensor_copy(
            v_sb[:, :, :, D:],
            ones_t[:, 0:1].broadcast_to([128, NT, H_LOC, D]),
        )

        LTt = {
            (h, cp): lt_pool.tile([128, 2, M], FR, name=f"lt_{h}_{cp}")
            for h in range(H_LOC)
            for cp in range(NCH)
        }

        pts = {}

        def u_S(p, c, jt):
            s_t = s_pool.tile([128, 1024], FP, tag="s", name="s_t")
            i0 = c * CHUNK
            for e in range(2):
                r0, r1 = e * 64, (e + 1) * 64
                nc.tensor.matmul(
                    s_t[:, e * 512 : (e + 1) * 512],
                    lhsT=kT[r0:r1, p, jt * 128 : (jt + 1) * 128],
                    rhs=qT[r0:r1, p, i0 : i0 + CHUNK],
                    start=True,
                    stop=True,
                    tile_position=(e * 64, 0),
                )
            pt = pt_pool.tile([128, 1024], BF, tag="pt", name="pt")
            nc.scalar.activation(out=pt, in_=s_t, func=Exp, scale=SCALE)
            pts[(p, c, jt)] = pt

        def u_PV(p, c, jt, o_t):
            pt = pts.pop((p, c, jt))
            for e in range(2):
                h = 2 * p + e
                nc.tensor.matmul(
                    o_t[:, e * 512 : (e + 1) * 512],
                    lhsT=v_sb[:, jt, h, :],
                    rhs=pt[:, e * 512 : (e + 1) * 512],
                    start=(jt == 0),
                    stop=(jt == NT - 1),
                )

        def u_norm(p, c, o_t, act_ou=False):
            for e in range(2):
                h = 2 * p + e
                ou_t = ou_pool.tile([128, 512], FP, tag="ou", name="ou_t")
                if act_ou:
                    nc.scalar.activation(
                        out=ou_t,
                        in_=o_t[:, e * 512 : (e + 1) * 512],
                        func=Copy,
                    )
                else:
                    nc.vector.tensor_copy(
                        ou_t, o_t[:, e * 512 : (e + 1) * 512]
                    )
                nc.vector.reciprocal(
                    out=ou_t[64:128, :], in_=ou_t[64:128, :]
                )
                rb_t = rb_pool.tile([64, 512], FP, tag="rb", name="rb_t")
                nc.sync.dma_start(out=rb_t, in_=ou_t[64:128, :])
                ou_w = ou_t.rearrange("q (w m) -> q w m", w=4)
                rb_w = rb_t.rearrange("q (w m) -> q w m", w=4)
                lodd_t = lodd_pool.tile([64, 2, M], FR, tag="lo", name="lo")
                nc.vector.tensor_mul(
                    LTt[(h, c)][0:64, :, :],
                    ou_w[0:64, 0::2, :],
                    rb_w[:, 0::2, :],
                )
                nc.vector.tensor_mul(
                    lodd_t,
                    ou_w[0:64, 1::2, :],
                    rb_w[:, 1::2, :],
                )
                nc.sync.dma_start(
                    out=LTt[(h, c)][64:128, :, :],
                    in_=lodd_t,
                )

        o_tiles = {}

        def blk(g):
            return (g // 64, (g // 16) % 4, g % 16)

        def run_stream(g0, g1, fillers):
            for g in range(g0, g1):
                if g + 1 < NB:
                    u_S(*blk(g + 1))
                for f in fillers.get(g, ()):
                    f()
                pv = g - LAG
                if 0 <= pv < NB:
                    p, c, jt = blk(pv)
                    if jt == 0:
                        o_tiles[(p, c)] = o_pool.tile(
                            [128, 1024], FP, tag="o", name="o_t"
                        )
                    u_PV(p, c, jt, o_tiles[(p, c)])
                    if jt == NT - 1:
                        u_norm(
                            p, c, o_tiles.pop((p, c)), act_ou=(pv == NB - 1)
                        )

        with ExitStack() as pctx:
            pro = pctx.enter_context(tc.tile_pool(name="pro", bufs=1))
            xpool = pctx.enter_context(tc.tile_pool(name="xtiles", bufs=8))

            ident = pro.tile([128, 128], FP)
            make_identity(nc, ident)
            xTr = pro.tile([128, CT, 2, CHUNK], FR)
            wq_sb = pro.tile([128, CT, INNER_LOC], FR)
            wk_sb = pro.tile([128, CT, INNER_LOC], FR)
            wv_sb = pro.tile([128, CT, INNER_LOC], FR)
            wraw_pool = pctx.enter_context(
                tc.tile_pool(name="wraw", bufs=2)
            )
            XSL = (0, 1, 0, 1)

            def u_wload(wdram, wsb):
                wr = wraw_pool.tile(
                    [128, CT, INNER_LOC], FP, tag="wr", name="wr"
                )
                nc.sync.dma_start(
                    out=wr, in_=wdram.rearrange("(t r) f -> r t f", r=128)
                )
                nc.scalar.activation(out=wsb, in_=wr, func=Copy)

            xts = {}

            def u_xload(ng):
                for k in range(4):
                    nt = ng * 4 + k
                    x_t = xpool.tile([128, C], FP, tag="x_t", name="x_t")
                    nc.sync.dma_start(
                        out=x_t, in_=x[nt * 128 : (nt + 1) * 128, :]
                    )
                    xts[nt] = x_t

            def u_T(ng, cts):
                for ct in cts:
                    ps_t = sc_pool.tile([128, 512], FP, tag="sc", name="ps_t")
                    for k in range(4):
                        nt = ng * 4 + k
                        nc.tensor.transpose(
                            ps_t[:, k * 128 : (k + 1) * 128],
                            xts[nt][:, ct * 128 : (ct + 1) * 128],
                            ident,
                        )
                    nc.vector.tensor_copy(xTr[:, ct, XSL[ng], :], ps_t)

            def u_QK(ng, p, dst, wsb):
                ps_p = sc_pool.tile([128, 512], FP, tag="sc", name="ps_p")
                for ct in range(CT):
                    nc.tensor.matmul(
                        ps_p,
                        lhsT=wsb[:, ct, p * 128 : (p + 1) * 128],
                        rhs=xTr[:, ct, XSL[ng], :],
                        start=(ct == 0),
                        stop=(ct == CT - 1),
                    )
                nc.vector.tensor_copy(
                    dst[:, p, ng * 512 : (ng + 1) * 512], ps_p
                )

            def u_V(ng, k):
                nt = ng * 4 + k
                ps_v = sc_pool.tile([128, 512], FP, tag="sc", name="ps_v")
                for ct in range(CT):
                    nc.tensor.matmul(
                        ps_v[:, 0:INNER_LOC],
                        lhsT=xTr[:, ct, XSL[ng], k * 128 : (k + 1) * 128],
                        rhs=wv_sb[:, ct, :],
                        start=(ct == 0),
                        stop=(ct == CT - 1),
                    )
                nc.vector.tensor_copy(
                    v_sb[:, nt, :, 0:D],
                    ps_v[:, 0:INNER_LOC].rearrange(
                        "p (h d) -> p h d", h=H_LOC
                    ),
                )

            u_xload(0)
            u_wload(wq, wq_sb)
            u_wload(wk, wk_sb)
            u_xload(1)
            u_wload(wv, wv_sb)
            nc.sync.dma_start(out=bo_raw, in_=bo)
            nc.vector.tensor_copy(bo_fr, bo_raw)

            def u_warm(n):
                wp = sc_pool.tile([128, 512], FP, tag="sc", name="wp")
                for w in range(n):
                    nc.tensor.matmul(
                        wp[:, 0:128],
                        lhsT=ones_fr,
                        rhs=ones_fr,
                        start=(w == 0),
                        stop=(w == n - 1),
                    )

            u_T(0, [0, 1, 2, 3])
            u_T(0, [4, 5, 6, 7])
            u_QK(0, 0, qT, wq_sb)
            u_QK(0, 0, kT, wk_sb)
            u_xload(2)
            u_S(0, 0, 0)

            pro_fillers = {
                1: [lambda: u_T(1, [0, 1, 2, 3])],
                2: [
                    lambda: u_T(1, [4, 5, 6, 7]),
                    lambda: u_QK(1, 0, kT, wk_sb),
                ],
                3: [
                    lambda: u_xload(3),
                    lambda: u_QK(0, 1, qT, wq_sb),
                ],
                4: [lambda: u_QK(0, 1, kT, wk_sb)],
                5: [
                    lambda: u_V(0, 0),
                    lambda: u_V(0, 1),
                    lambda: u_V(0, 2),
                    lambda: u_V(0, 3),
                ],
                6: [
                    lambda: u_T(2, [0, 1, 2, 3]),
                    lambda: u_T(2, [4, 5, 6, 7]),
                    lambda: u_QK(2, 0, kT, wk_sb),
                ],
                8: [
                    lambda: u_V(1, 0),
                    lambda: u_V(1, 1),
                    lambda: u_QK(1, 0, qT, wq_sb),
                ],
                9: [
                    lambda: u_V(1, 2),
                    lambda: u_V(1, 3),
                    lambda: u_QK(1, 1, qT, wq_sb),
                    lambda: u_QK(1, 1, kT, wk_sb),
                ],
                10: [
                    lambda: u_T(3, [0, 1, 2, 3]),
                    lambda: u_T(3, [4, 5, 6, 7]),
                    lambda: u_QK(3, 0, kT, wk_sb),
                ],
                11: [lambda: u_V(2, 0), lambda: u_V(2, 1)],
                12: [lambda: u_V(2, 2), lambda: u_V(2, 3)],
                13: [lambda: u_V(3, 0), lambda: u_V(3, 1)],
                14: [lambda: u_V(3, 2)],
                15: [lambda: u_V(3, 3)],
                20: [lambda: u_QK(2, 0, qT, wq_sb)],
                27: [lambda: u_QK(2, 1, qT, wq_sb)],
                33: [lambda: u_QK(2, 1, kT, wk_sb)],
                40: [lambda: u_QK(3, 0, qT, wq_sb)],
                47: [lambda: u_QK(3, 1, qT, wq_sb)],
                54: [lambda: u_QK(3, 1, kT, wk_sb)],
            }
            run_stream(0, 57, pro_fillers)

        wo_pool = ctx.enter_context(tc.tile_pool(name="wo_sbuf", bufs=1))
        wo_ring = ctx.enter_context(tc.tile_pool(name="wo_ring", bufs=2))
        wo_sb = wo_pool.tile([128, CT, C], FR)
        for kt in range(CT):
            wo_raw = wo_ring.tile([128, C], FP, tag="wor", name="wo_raw")
            nc.sync.dma_start(
                out=wo_raw, in_=wo[kt * 128 : (kt + 1) * 128, :]
            )
            nc.vector.tensor_copy(wo_sb[:, kt, :], wo_raw)

        def u_Fk(h, oc, kt, f_ps, first=False):
            return nc.tensor.matmul(
                f_ps,
                lhsT=LTt[(h, kt // 2)][:, kt % 2, :],
                rhs=wo_sb[:, kt, oc * 512 : (oc + 1) * 512],
                start=first,
                stop=False,
            )

        def u_Fend(h, oc, f_ps, dve=False):
            nc.tensor.matmul(
                f_ps,
                lhsT=ones_fr,
                rhs=bo_fr[0:1, oc * 512 : (oc + 1) * 512],
                start=False,
                stop=True,
            )
            ob = out_pool.tile([128, 512], FP, tag="ob", name="ob")
            if dve:
                nc.vector.tensor_copy(ob, f_ps)
                nc.scalar.dma_start(
                    out=out[h, :, oc * 512 : (oc + 1) * 512], in_=ob
                )
            else:
                nc.scalar.activation(out=ob, in_=f_ps, func=Copy)
                nc.sync.dma_start(
                    out=out[h, :, oc * 512 : (oc + 1) * 512], in_=ob
                )

        f_tiles = {}

        def f_ps_of(h, oc):
            if (h, oc) not in f_tiles:
                f_tiles[(h, oc)] = sc_pool.tile(
                    [128, 512], FP, tag="sc", name=f"f_{h}_{oc}"
                )
            return f_tiles[(h, oc)]

        def fk(h, oc, kt, first=False):
            return lambda: u_Fk(h, oc, kt, f_ps_of(h, oc), first)

        def fend(h, oc, dve=False):
            return lambda: u_Fend(h, oc, f_ps_of(h, oc), dve=dve)

        def _g0(kt):
            return 16 * (kt // 2 + 1) + LAG + 1

        chain_a = (
            [(fk(0, 0, kt, kt == 0), _g0(kt)) for kt in range(8)]
            + [(fend(0, 0), _g0(7))]
            + [(fk(1, 0, kt, kt == 0), _g0(kt)) for kt in range(8)]
            + [(fend(1, 0, True), _g0(7))]
        )
        chain_b = (
            [(fk(0, 1, kt, kt == 0), _g0(kt)) for kt in range(8)]
            + [(fend(0, 1), _g0(7))]
            + [(fk(1, 1, kt, kt == 0), _g0(kt)) for kt in range(8)]
            + [(fend(1, 1, True), _g0(7))]
        )
        reserve_a = [chain_a.pop()[0] for _ in range(4)][::-1]
        reserve_b = [chain_b.pop()[0] for _ in range(4)][::-1]
        p1_fillers = {}
        turn = [chain_a, chain_b]
        for g in range(57, 128):
            lst = []
            for chain in turn:
                if len(lst) >= (2 if g % 16 >= 14 else 1):
                    break
                if chain and chain[0][1] <= g:
                    lst.append(chain.pop(0)[0])
            turn.reverse()
            if lst:
                p1_fillers[g] = lst

        f3ab = {}

        def f3_ps(h3):
            if h3 not in f3ab:
                t = s_pool.tile(
                    [128, 1024], FP, tag="s", name=f"f3_{h3}"
                )
                f3ab[h3] = t[:, 0:512]
                f_tiles[(3, h3)] = f3ab[h3]
            return f3ab[h3]

        def f3k(oc, kt, first=False):
            return lambda: u_Fk(3, oc, kt, f3_ps(oc), first)

        def f2k(oc, kt, first=False):
            return lambda: u_Fk(2, oc, kt, f_ps_of(2, oc), first)

        tail_ready = (
            reserve_a
            + reserve_b
            + [f2k(0, 0, True), f2k(1, 0, True), f2k(0, 1), f2k(1, 1)]
            + [f2k(0, 2), f2k(1, 2), f2k(0, 3), f2k(1, 3)]
            + [f2k(0, 4), f2k(1, 4), f2k(0, 5), f2k(1, 5)]
            + [f3k(0, 0, True), f3k(1, 0, True), f3k(0, 1), f3k(1, 1)]
            + [f3k(0, 2), f3k(1, 2), f3k(0, 3), f3k(1, 3)]
            + [f3k(0, 4), f3k(1, 4), f3k(0, 5), f3k(1, 5)]
        )
        for g in range(128, 128 + LAG):
            p1_fillers[g] = [tail_ready.pop(0) for _ in range(4)
                             if tail_ready]
        run_stream(57, NB + LAG, p1_fillers)

        for fcl in tail_ready:
            fcl()
        warm_ps = o_pool.tile([128, 1024], FP, tag="o", name="warm_ps")
        NWARM = 24
        for w in range(NWARM):
            nc.tensor.matmul(
                warm_ps[:, 0:512],
                lhsT=ones_fr,
                rhs=bo_fr[0:1, 0:512],
                start=(w == 0),
                stop=(w == NWARM - 1),
            )
        last4 = ((2, 0, False), (2, 1, True), (3, 0, True), (3, 1, False))
        for h, oc, _ in last4:
            u_Fk(h, oc, 6, f_tiles[(h, oc)])
        for h, oc, dve in last4:
            u_Fk(h, oc, 7, f_tiles[(h, oc)])
            u_Fend(h, oc, f_tiles[(h, oc)], dve=dve)

_NC = None


def _get_nc():
    global _NC
    if _NC is None:
        _NC = _build_kernel()
    return _NC


def _make_in_maps(x, Wq, Wkv, Wo, bo):
    in_maps = []
    for c in range(N_CORES):
        b = c // 4
        g = c % 4
        cols = slice(g * INNER_LOC, (g + 1) * INNER_LOC)
        in_maps.append(
            {
                "x": np.ascontiguousarray(x[b]),
                "wq": np.ascontiguousarray(Wq[:, cols]),
                "wk": np.ascontiguousarray(Wkv[:, cols]),
                "wv": np.ascontiguousarray(
                    Wkv[:, C + g * INNER_LOC : C + (g + 1) * INNER_LOC]
                ),
                "wo": np.ascontiguousarray(Wo),
                "bo": np.ascontiguousarray(bo.reshape(1, C)),
            }
        )
    return in_maps


def _run(x, Wq, Wkv, Wo, bo, **run_kwargs):
    nc = _get_nc()
    in_maps = _make_in_maps(x, Wq, Wkv, Wo, bo)
    res = bass_utils.run_bass_kernel_spmd(
        nc, in_maps, core_ids=list(range(N_CORES)), **run_kwargs
    )
    outs = [res.results[c]["out"].reshape(H_LOC, M, C) for c in range(N_CORES)]
    full = np.concatenate(outs, axis=0).astype(np.float32)
    return full, res


def kernel(x, Wq, Wkv, Wo, bo):
    x = np.asarray(x, dtype=np.float32)
    Wq = np.asarray(Wq, dtype=np.float32)
    Wkv = np.asarray(Wkv, dtype=np.float32)
    Wo = np.asarray(Wo, dtype=np.float32)
    bo = np.asarray(bo, dtype=np.float32)
    full, _ = _run(x, Wq, Wkv, Wo, bo)
    return full


# revision 44
# speedup vs baseline: 1.2395x; 1.0046x over previous
import os
import sys
from contextlib import ExitStack

import numpy as np

for _p in ("/opt/trn_rl_repo", "/root/.axon_site/_ro/trn_rl_repo"):
    if os.path.isdir(_p) and _p not in sys.path:
        sys.path.insert(0, _p)

import concourse.bass as bass
import concourse.tile as tile
from concourse import bacc
from concourse import mybir
from concourse import bass_utils
from concourse.masks import make_identity

N_CORES = 8
B = 2
N = 2048
C = 1024
H_TOT = 16
D = 64
H_LOC = 4
PAIRS = 2
INNER_LOC = H_LOC * D
M = N // H_TOT
CT = C // 128
NT = N // 128
NCH = 4
CHUNK = 512
SCALE = D ** -0.5
FP = mybir.dt.float32
FR = mybir.dt.float32r


def _r(ap):
    return ap.bitcast(FR)


def _build_kernel():
    nc = bacc.Bacc("TRN2", target_bir_lowering=False, debug=False)
    x = nc.dram_tensor("x", (N, C), FP, kind="ExternalInput").ap()
    wq = nc.dram_tensor("wq", (C, INNER_LOC), FP, kind="ExternalInput").ap()
    wk = nc.dram_tensor("wk", (C, INNER_LOC), FP, kind="ExternalInput").ap()
    wv = nc.dram_tensor("wv", (C, INNER_LOC), FP, kind="ExternalInput").ap()
    wo = nc.dram_tensor("wo", (C, C), FP, kind="ExternalInput").ap()
    bo = nc.dram_tensor("bo", (1, C), FP, kind="ExternalInput").ap()
    out = nc.dram_tensor("out", (H_LOC, M, C), FP, kind="ExternalOutput").ap()

    with tile.TileContext(nc) as tc:
        _trace_kernel(tc, out, x, wq, wk, wv, wo, bo)
    nc.compile()
    return nc


def _trace_kernel(tc, out, x, wq, wk, wv, wo, bo):
    nc = tc.nc
    Exp = mybir.ActivationFunctionType.Exp
    Copy = mybir.ActivationFunctionType.Copy
    BF = mybir.dt.bfloat16
    LAG = 6
    NB = 128

    with ExitStack() as ctx:
        consts = ctx.enter_context(tc.tile_pool(name="consts", bufs=1))
        qkv_pool = ctx.enter_context(tc.tile_pool(name="qkv", bufs=1))
        lt_pool = ctx.enter_context(tc.tile_pool(name="lt", bufs=1))
        lodd_pool = ctx.enter_context(tc.tile_pool(name="lodd", bufs=1))
        pt_pool = ctx.enter_context(tc.tile_pool(name="pt", bufs=LAG + 2))
        ou_pool = ctx.enter_context(tc.tile_pool(name="ou", bufs=2))
        rb_pool = ctx.enter_context(tc.tile_pool(name="rb", bufs=2))
        out_pool = ctx.enter_context(tc.tile_pool(name="outsb", bufs=2))

        s_pool = ctx.enter_context(
            tc.tile_pool(name="s_ps", bufs=2, space="PSUM")
        )
        o_pool = ctx.enter_context(
            tc.tile_pool(name="o_ps", bufs=1, space="PSUM")
        )
        sc_pool = ctx.enter_context(
            tc.tile_pool(name="sc_ps", bufs=2, space="PSUM")
        )

        ones_t = consts.tile([128, 128], FP)
        nc.vector.memset(ones_t, 1.0)
        bo_raw = consts.tile([1, C], FP)
        bo_fr = consts.tile([1, C], FR)
        ones_fr = consts.tile([1, 128], FR)
        nc.vector.tensor_copy(ones_fr, ones_t[0:1, :])
        warm = consts.tile([1, 128], FR)
        nc.scalar.activation(out=warm, in_=ones_t[0:1, :], func=Exp, scale=1.0)

        qT = qkv_pool.tile([128, PAIRS, N], FR)
        kT = qkv_pool.tile([128, PAIRS, N], FR)
        v_sb = qkv_pool.tile([128, NT, H_LOC, 128], BF)
        nc.vector.tensor_copy(
            v_sb[:, :, :, D:],
            ones_t[:, 0:1].broadcast_to([128, NT, H_LOC, D]),
        )

        LTt = {
            (h, cp): lt_pool.tile([128, 2, M], FR, name=f"lt_{h}_{cp}")
            for h in range(H_LOC)
            for cp in range(NCH)
        }

        pts = {}

        def u_S(p, c, jt):
            s_t = s_pool.tile([128, 1024], FP, tag="s", name="s_t")
            i0 = c * CHUNK
            for e in range(2):
                r0, r1 = e * 64, (e + 1) * 64
                nc.tensor.matmul(
                    s_t[:, e * 512 : (e + 1) * 512],
                    lhsT=kT[r0:r1, p, jt * 128 : (jt + 1) * 128],
                    rhs=qT[r0:r1, p, i0 : i0 + CHUNK],
                    start=True,
                    stop=True,
                    tile_position=(e * 64, 0),
                )
            pt = pt_pool.tile([128, 1024], BF, tag="pt", name="pt")
            nc.scalar.activation(out=pt, in_=s_t, func=Exp, scale=SCALE)
            pts[(p, c, jt)] = pt

        def u_PV(p, c, jt, o_t):
            pt = pts.pop((p, c, jt))
            for e in range(2):
                h = 2 * p + e
                nc.tensor.matmul(
                    o_t[:, e * 512 : (e + 1) * 512],
                    lhsT=v_sb[:, jt, h, :],
                    rhs=pt[:, e * 512 : (e + 1) * 512],
                    start=(jt == 0),
                    stop=(jt == NT - 1),
                )

        def u_norm(p, c, o_t, act_ou=False):
            for e in range(2):
                h = 2 * p + e
                ou_t = ou_pool.tile([128, 512], FP, tag="ou", name="ou_t")
                if act_ou:
                    nc.scalar.activation(
                        out=ou_t,
                        in_=o_t[:, e * 512 : (e + 1) * 512],
                        func=Copy,
                    )
                else:
                    nc.vector.tensor_copy(
                        ou_t, o_t[:, e * 512 : (e + 1) * 512]
                    )
                nc.vector.reciprocal(
                    out=ou_t[64:128, :], in_=ou_t[64:128, :]
                )
                rb_t = rb_pool.tile([64, 512], FP, tag="rb", name="rb_t")
                nc.sync.dma_start(out=rb_t, in_=ou_t[64:128, :])
                ou_w = ou_t.rearrange("q (w m) -> q w m", w=4)
                rb_w = rb_t.rearrange("q (w m) -> q w m", w=4)
                lodd_t = lodd_pool.tile([64, 2, M], FR, tag="lo", name="lo")
                nc.vector.tensor_mul(
                    LTt[(h, c)][0:64, :, :],
                    ou_w[0:64, 0::2, :],
                    rb_w[:, 0::2, :],
                )
                nc.vector.tensor_mul(
                    lodd_t,
                    ou_w[0:64, 1::2, :],
                    rb_w[:, 1::2, :],
                )
                nc.sync.dma_start(
                    out=LTt[(h, c)][64:128, :, :],
                    in_=lodd_t,
                )

        o_tiles = {}

        def blk(g):
            return (g // 64, (g // 16) % 4, g % 16)

        def run_stream(g0, g1, fillers):
            for g in range(g0, g1):
                if g + 1 < NB:
                    u_S(*blk(g + 1))
                for f in fillers.get(g, ()):
                    f()
                pv = g - LAG
                if 0 <= pv < NB:
                    p, c, jt = blk(pv)
                    if jt == 0:
                        o_tiles[(p, c)] = o_pool.tile(
                            [128, 1024], FP, tag="o", name="o_t"
                        )
                    u_PV(p, c, jt, o_tiles[(p, c)])
                    if jt == NT - 1:
                        u_norm(
                            p, c, o_tiles.pop((p, c)), act_ou=(pv == NB - 1)
                        )

        with ExitStack() as pctx:
            pro = pctx.enter_context(tc.tile_pool(name="pro", bufs=1))
            xpool = pctx.enter_context(tc.tile_pool(name="xtiles", bufs=8))

            ident = pro.tile([128, 128], FP)
            make_identity(nc, ident)
            xTr = pro.tile([128, CT, 2, CHUNK], FR)
            wq_sb = pro.tile([128, CT, INNER_LOC], FR)
            wk_sb = pro.tile([128, CT, INNER_LOC], FR)
            wv_sb = pro.tile([128, CT, INNER_LOC], FR)
            wraw_pool = pctx.enter_context(
                tc.tile_pool(name="wraw", bufs=2)
            )
            XSL = (0, 1, 0, 1)

            def u_wload(wdram, wsb, dve=False):
                wr = wraw_pool.tile(
                    [128, CT, INNER_LOC], FP, tag="wr", name="wr"
                )
                nc.sync.dma_start(
                    out=wr, in_=wdram.rearrange("(t r) f -> r t f", r=128)
                )
                if dve:
                    nc.vector.tensor_copy(wsb, wr)
                else:
                    nc.scalar.activation(out=wsb, in_=wr, func=Copy)

            xts = {}

            def u_xload(ng):
                for k in range(4):
                    nt = ng * 4 + k
                    x_t = xpool.tile([128, C], FP, tag="x_t", name="x_t")
                    nc.sync.dma_start(
                        out=x_t, in_=x[nt * 128 : (nt + 1) * 128, :]
                    )
                    xts[nt] = x_t

            def u_T(ng, cts):
                for ct in cts:
                    ps_t = sc_pool.tile([128, 512], FP, tag="sc", name="ps_t")
                    for k in range(4):
                        nt = ng * 4 + k
                        nc.tensor.transpose(
                            ps_t[:, k * 128 : (k + 1) * 128],
                            xts[nt][:, ct * 128 : (ct + 1) * 128],
                            ident,
                        )
                    nc.vector.tensor_copy(xTr[:, ct, XSL[ng], :], ps_t)

            def u_QK(ng, p, dst, wsb):
                ps_p = sc_pool.tile([128, 512], FP, tag="sc", name="ps_p")
                for ct in range(CT):
                    nc.tensor.matmul(
                        ps_p,
                        lhsT=wsb[:, ct, p * 128 : (p + 1) * 128],
                        rhs=xTr[:, ct, XSL[ng], :],
                        start=(ct == 0),
                        stop=(ct == CT - 1),
                    )
                nc.vector.tensor_copy(
                    dst[:, p, ng * 512 : (ng + 1) * 512], ps_p
                )

            def u_V(ng, k):
                nt = ng * 4 + k
                ps_v = sc_pool.tile([128, 512], FP, tag="sc", name="ps_v")
                for ct in range(CT):
                    nc.tensor.matmul(
                        ps_v[:, 0:INNER_LOC],
                        lhsT=xTr[:, ct, XSL[ng], k * 128 : (k + 1) * 128],
                        rhs=wv_sb[:, ct, :],
                        start=(ct == 0),
                        stop=(ct == CT - 1),
                    )
                nc.vector.tensor_copy(
                    v_sb[:, nt, :, 0:D],
                    ps_v[:, 0:INNER_LOC].rearrange(
                        "p (h d) -> p h d", h=H_LOC
                    ),
                )

            u_xload(0)
            u_wload(wq, wq_sb)
            u_wload(wk, wk_sb)
            u_xload(1)
            u_wload(wv, wv_sb)
            nc.sync.dma_start(out=bo_raw, in_=bo)
            nc.vector.tensor_copy(bo_fr, bo_raw)

            def u_warm(n):
                wp = sc_pool.tile([128, 512], FP, tag="sc", name="wp")
                for w in range(n):
                    nc.tensor.matmul(
                        wp[:, 0:128],
                        lhsT=ones_fr,
                        rhs=ones_fr,
                        start=(w == 0),
                        stop=(w == n - 1),
                    )

            u_T(0, [0, 1, 2, 3])
            u_T(0, [4, 5, 6, 7])
            u_QK(0, 0, qT, wq_sb)
            u_QK(0, 0, kT, wk_sb)
            u_xload(2)
            u_S(0, 0, 0)

            pro_fillers = {
                1: [lambda: u_T(1, [0, 1, 2, 3])],
                2: [
                    lambda: u_T(1, [4, 5, 6, 7]),
                    lambda: u_QK(1, 0, kT, wk_sb),
                ],
                3: [
                    lambda: u_xload(3),
                    lambda: u_QK(0, 1, qT, wq_sb),
                ],
                4: [lambda: u_QK(0, 1, kT, wk_sb)],
                5: [
                    lambda: u_V(0, 0),
                    lambda: u_V(0, 1),
                    lambda: u_V(0, 2),
                    lambda: u_V(0, 3),
                ],
                6: [
                    lambda: u_T(2, [0, 1, 2, 3]),
                    lambda: u_T(2, [4, 5, 6, 7]),
                    lambda: u_QK(2, 0, kT, wk_sb),
                ],
                8: [
                    lambda: u_V(1, 0),
                    lambda: u_V(1, 1),
                    lambda: u_QK(1, 0, qT, wq_sb),
                ],
                9: [
                    lambda: u_V(1, 2),
                    lambda: u_V(1, 3),
                    lambda: u_QK(1, 1, qT, wq_sb),
                    lambda: u_QK(1, 1, kT, wk_sb),
                ],
                10: [
                    lambda: u_T(3, [0, 1, 2, 3]),
                    lambda: u_T(3, [4, 5, 6, 7]),
                    lambda: u_QK(3, 0, kT, wk_sb),
                ],
                11: [lambda: u_V(2, 0), lambda: u_V(2, 1)],
                12: [lambda: u_V(2, 2), lambda: u_V(2, 3)],
                13: [lambda: u_V(3, 0), lambda: u_V(3, 1)],
                14: [lambda: u_V(3, 2)],
                15: [lambda: u_V(3, 3)],
                20: [lambda: u_QK(2, 0, qT, wq_sb)],
                27: [lambda: u_QK(2, 1, qT, wq_sb)],
                33: [lambda: u_QK(2, 1, kT, wk_sb)],
                40: [lambda: u_QK(3, 0, qT, wq_sb)],
                47: [lambda: u_QK(3, 1, qT, wq_sb)],
                54: [lambda: u_QK(3, 1, kT, wk_sb)],
            }
            run_stream(0, 57, pro_fillers)

        wo_pool = ctx.enter_context(tc.tile_pool(name="wo_sbuf", bufs=1))
        wo_ring = ctx.enter_context(tc.tile_pool(name="wo_ring", bufs=2))
        wo_sb = wo_pool.tile([128, CT, C], FR)
        for kt in range(CT):
            wo_raw = wo_ring.tile([128, C], FP, tag="wor", name="wo_raw")
            nc.sync.dma_start(
                out=wo_raw, in_=wo[kt * 128 : (kt + 1) * 128, :]
            )
            nc.vector.tensor_copy(wo_sb[:, kt, :], wo_raw)

        def u_Fk(h, oc, kt, f_ps, first=False):
            return nc.tensor.matmul(
                f_ps,
                lhsT=LTt[(h, kt // 2)][:, kt % 2, :],
                rhs=wo_sb[:, kt, oc * 512 : (oc + 1) * 512],
                start=first,
                stop=False,
            )

        def u_Fend(h, oc, f_ps, dve=False):
            nc.tensor.matmul(
                f_ps,
                lhsT=ones_fr,
                rhs=bo_fr[0:1, oc * 512 : (oc + 1) * 512],
                start=False,
                stop=True,
            )
            ob = out_pool.tile([128, 512], FP, tag="ob", name="ob")
            if dve:
                nc.vector.tensor_copy(ob, f_ps)
                nc.scalar.dma_start(
                    out=out[h, :, oc * 512 : (oc + 1) * 512], in_=ob
                )
            else:
                nc.scalar.activation(out=ob, in_=f_ps, func=Copy)
                nc.sync.dma_start(
                    out=out[h, :, oc * 512 : (oc + 1) * 512], in_=ob
                )

        f_tiles = {}

        def f_ps_of(h, oc):
            if (h, oc) not in f_tiles:
                f_tiles[(h, oc)] = sc_pool.tile(
                    [128, 512], FP, tag="sc", name=f"f_{h}_{oc}"
                )
            return f_tiles[(h, oc)]

        def fk(h, oc, kt, first=False):
            return lambda: u_Fk(h, oc, kt, f_ps_of(h, oc), first)

        def fend(h, oc, dve=False):
            return lambda: u_Fend(h, oc, f_ps_of(h, oc), dve=dve)

        def _g0(kt):
            return 16 * (kt // 2 + 1) + LAG + 1

        chain_a = (
            [(fk(0, 0, kt, kt == 0), _g0(kt)) for kt in range(8)]
            + [(fend(0, 0, True), _g0(7))]
            + [(fk(1, 0, kt, kt == 0), _g0(kt)) for kt in range(8)]
            + [(fend(1, 0, True), _g0(7))]
        )
        chain_b = (
            [(fk(0, 1, kt, kt == 0), _g0(kt)) for kt in range(8)]
            + [(fend(0, 1, True), _g0(7))]
            + [(fk(1, 1, kt, kt == 0), _g0(kt)) for kt in range(8)]
            + [(fend(1, 1, True), _g0(7))]
        )
        reserve_a = [chain_a.pop()[0] for _ in range(4)][::-1]
        reserve_b = [chain_b.pop()[0] for _ in range(4)][::-1]
        p1_fillers = {}
        turn = [chain_a, chain_b]
        for g in range(57, 128):
            lst = []
            for chain in turn:
                if len(lst) >= (2 if g % 16 >= 14 else 1):
                    break
                if chain and chain[0][1] <= g:
                    lst.append(chain.pop(0)[0])
            turn.reverse()
            if lst:
                p1_fillers[g] = lst

        f3ab = {}

        def f3_ps(h3):
            if h3 not in f3ab:
                t = s_pool.tile(
                    [128, 1024], FP, tag="s", name=f"f3_{h3}"
                )
                f3ab[h3] = t[:, 0:512]
                f_tiles[(3, h3)] = f3ab[h3]
            return f3ab[h3]

        def f3k(oc, kt, first=False):
            return lambda: u_Fk(3, oc, kt, f3_ps(oc), first)

        def f2k(oc, kt, first=False):
            return lambda: u_Fk(2, oc, kt, f_ps_of(2, oc), first)

        tail_ready = (
            reserve_a
            + reserve_b
            + [f2k(0, 0, True), f2k(1, 0, True), f2k(0, 1), f2k(1, 1)]
            + [f2k(0, 2), f2k(1, 2), f2k(0, 3), f2k(1, 3)]
            + [f2k(0, 4), f2k(1, 4), f2k(0, 5), f2k(1, 5)]
            + [f3k(0, 0, True), f3k(1, 0, True), f3k(0, 1), f3k(1, 1)]
            + [f3k(0, 2), f3k(1, 2), f3k(0, 3), f3k(1, 3)]
            + [f3k(0, 4), f3k(1, 4), f3k(0, 5), f3k(1, 5)]
        )
        for g in range(128, 128 + LAG):
            p1_fillers[g] = [tail_ready.pop(0) for _ in range(4)
                             if tail_ready]
        run_stream(57, NB + LAG, p1_fillers)

        for fcl in tail_ready:
            fcl()
        warm_ps = o_pool.tile([128, 1024], FP, tag="o", name="warm_ps")
        NWARM = 24
        for w in range(NWARM):
            nc.tensor.matmul(
                warm_ps[:, 0:512],
                lhsT=ones_fr,
                rhs=bo_fr[0:1, 0:512],
                start=(w == 0),
                stop=(w == NWARM - 1),
            )
        last4 = ((2, 0, False), (2, 1, True), (3, 0, True), (3, 1, False))
        for h, oc, _ in last4:
            u_Fk(h, oc, 6, f_tiles[(h, oc)])
        for h, oc, dve in last4:
            u_Fk(h, oc, 7, f_tiles[(h, oc)])
            u_Fend(h, oc, f_tiles[(h, oc)], dve=dve)

_NC = None


def _get_nc():
    global _NC
    if _NC is None:
        _NC = _build_kernel()
    return _NC


def _make_in_maps(x, Wq, Wkv, Wo, bo):
    in_maps = []
    for c in range(N_CORES):
        b = c // 4
        g = c % 4
        cols = slice(g * INNER_LOC, (g + 1) * INNER_LOC)
        in_maps.append(
            {
                "x": np.ascontiguousarray(x[b]),
                "wq": np.ascontiguousarray(Wq[:, cols]),
                "wk": np.ascontiguousarray(Wkv[:, cols]),
                "wv": np.ascontiguousarray(
                    Wkv[:, C + g * INNER_LOC : C + (g + 1) * INNER_LOC]
                ),
                "wo": np.ascontiguousarray(Wo),
                "bo": np.ascontiguousarray(bo.reshape(1, C)),
            }
        )
    return in_maps


def _run(x, Wq, Wkv, Wo, bo, **run_kwargs):
    nc = _get_nc()
    in_maps = _make_in_maps(x, Wq, Wkv, Wo, bo)
    res = bass_utils.run_bass_kernel_spmd(
        nc, in_maps, core_ids=list(range(N_CORES)), **run_kwargs
    )
    outs = [res.results[c]["out"].reshape(H_LOC, M, C) for c in range(N_CORES)]
    full = np.concatenate(outs, axis=0).astype(np.float32)
    return full, res


def kernel(x, Wq, Wkv, Wo, bo):
    x = np.asarray(x, dtype=np.float32)
    Wq = np.asarray(Wq, dtype=np.float32)
    Wkv = np.asarray(Wkv, dtype=np.float32)
    Wo = np.asarray(Wo, dtype=np.float32)
    bo = np.asarray(bo, dtype=np.float32)
    full, _ = _run(x, Wq, Wkv, Wo, bo)
    return full
